# revision 31
# baseline (speedup 1.0000x reference)
"""Cross-attention Trainium2 kernel, SPMD over 8 NeuronCores.

Sharding: core c = b*4 + g handles batch b (of 2) and head-group g (of 4,
4 heads each) — data parallel on B, Megatron tensor parallel on heads:
W_qkv column-sliced, W_proj row-sliced, partial outputs summed on host.

Per-core dataflow (all matmuls 16-bit operands, fp32 PSUM accumulation):
  - host pre-transposes x/context to [D, L] and pre-casts weights to bf16
  - Q^T[n,tok] = Wq.T @ x^T      (lhsT=Wq tile, rhs=x^T tile, + bias via ACT)
  - K^T[n,key] = Wk.T @ c^T      (same)
  - V[key,n]   = c @ Wv          (lhsT=c^T tile, rhs=Wv, bias via K=1 matmul)
  - S^T[key,tok] = K_h @ Q_h^T   (one matmul per tile, contraction=head_dim)
  - P^T = exp(scale * S^T)       (ACT, fp16 out; no max subtraction: scores
                                  ~ N(0,1) so exp never overflows)
  - O^T[hd,tok] = V_h.T @ P^T    (lhsT=V tile (stationary), rhs=P^T streams
                                  N=512 — avoids per-matmul LDW stalls and
                                  produces O^T directly: no PE transposes)
  - denom[tok] = ones.T @ (tree-sum of P^T tiles)  (DVE fp16 add tree +
                                  one K=128 matmul; reciprocal on DVE;
                                  broadcast across partitions on GPSIMD)
  - OT = O^T * recip(denom)      (DVE multiply fused with PSUM->SBUF copy)
  - out[tok,dout] = O @ Wp       (lhsT=O^T tile, rhs=Wp tile; bf16 DMA out)
"""

import numpy as np
import ml_dtypes

import concourse.bass as bass
import concourse.bacc as bacc
import concourse.mybir as mybir
from concourse.bass import ts
from concourse.tile import TileContext

DIM = 2048
NUM_HEADS = 16
HEAD_DIM = 128
B, L = 2, 2048
GPB = 4                  # head-groups per batch (cores per batch)
HPC = NUM_HEADS // GPB   # heads per core = 4
NPC = HPC * HEAD_DIM     # per-core projection width = 512
N_CORES = 8

F32 = mybir.dt.float32
BF16 = mybir.dt.bfloat16
FP16 = mybir.dt.float16
BF16_NP = ml_dtypes.bfloat16


def build_bass(dim=DIM, seq=L, hpc=HPC, hd=HEAD_DIM, repeat=1):
    """Build the per-core SPMD Bass program (parameterized for testing)."""
    npc = hpc * hd
    KT = dim // 128      # contraction tiles over model dim
    LT = seq // 128      # token/key 128-tiles
    TCH = seq // 512     # token 512-chunks
    scale = float(hd) ** -0.5
    Exp = mybir.ActivationFunctionType.Exp

    nc = bacc.Bacc()
    xT = nc.dram_tensor("xT", [dim, seq], BF16, kind="ExternalInput")
    cT = nc.dram_tensor("cT", [dim, seq], BF16, kind="ExternalInput")
    wq = nc.dram_tensor("wq", [dim, npc], BF16, kind="ExternalInput")
    wk = nc.dram_tensor("wk", [dim, npc], BF16, kind="ExternalInput")
    wv = nc.dram_tensor("wv", [dim, npc], BF16, kind="ExternalInput")
    bq = nc.dram_tensor("bq", [128, hpc], F32, kind="ExternalInput")
    bk = nc.dram_tensor("bk", [128, hpc], F32, kind="ExternalInput")
    bv = nc.dram_tensor("bv", [1, npc], F32, kind="ExternalInput")
    wp = nc.dram_tensor("wp", [npc, dim], BF16, kind="ExternalInput")
    out = nc.dram_tensor("out", [seq, dim], BF16, kind="ExternalOutput")

    with TileContext(nc) as tc:
        with (
            tc.tile_pool(name="psmm", bufs=5, space="PSUM") as psmm,
            tc.tile_pool(name="psov", bufs=2, space="PSUM") as psov,
            tc.tile_pool(name="psdn", bufs=1, space="PSUM") as psdn,
            tc.tile_pool(name="small", bufs=3) as small,
        ):
            for _rep in range(repeat):
                _build_body(
                    nc, tc, psmm, psov, psdn, small,
                    xT, cT, wq, wk, wv, bq, bk, bv, wp, out,
                    dim, seq, hpc, hd, npc, KT, LT, TCH, scale, Exp,
                )

    nc.compile()
    return nc


def _build_body(
    nc, tc, psmm, psov, psdn, small,
    xT, cT, wq, wk, wv, bq, bk, bv, wp, out,
    dim, seq, hpc, hd, npc, KT, LT, TCH, scale, Exp,
):
    with tc.tile_pool(name="res", bufs=1) as res:
        QT = res.tile([128, hpc, seq], BF16)    # [hd, h, tok]
        KTl = res.tile([128, hpc, seq], BF16)   # [hd, h, key]
        V = res.tile([128, LT, npc], BF16)      # [key, ktile, h*hd]
        OT = res.tile([128, hpc, seq], BF16)    # [hd, h, tok] normalized
        Wp_sb = res.tile([128, hpc, dim], BF16)
        bq_sb = res.tile([128, hpc], F32)
        bk_sb = res.tile([128, hpc], F32)
        bv_sb = res.tile([1, npc], F32)
        ones_col = res.tile([128, 1], FP16)     # denominator matmul stationary
        BV = res.tile([128, npc], F32)          # bv broadcast across partitions

        nc.vector.memset(ones_col[:], 1.0)

        # ---- phase 1: projections ----
        # stream + Wq pools stay alive through attention (Q chunks 1-3 are
        # computed inside the attention loop, in PE slack); Wk/Wv free early.
        with (
            tc.tile_pool(name="stream", bufs=2) as stream_pool,
            tc.tile_pool(name="wqpool", bufs=1) as wqpool,
        ):
            Wq_sb = wqpool.tile([128, KT, npc], BF16)
            wk_r = wk[:, :].rearrange("(kt p) n -> p kt n", p=128)
            wv_r = wv[:, :].rearrange("(kt p) n -> p kt n", p=128)
            wq_r = wq[:, :].rearrange("(kt p) n -> p kt n", p=128)

            def stream_chunk(src, t):
                st_tile = stream_pool.tile([128, KT, 512], BF16, tag="stream")
                src_r = src[:, ts(t, 512)].rearrange("(kt p) n -> p kt n", p=128)
                nc.sync.dma_start(st_tile[:], src_r[:, :, :])
                return st_tile

            # load order: sync queue carries biases + the ctx/x token streams,
            # scalar (ACT hwdge) queue carries the big weights — the two
            # queues drain concurrently so chunk-0 compute isn't serialized
            # behind 4MB of weights
            wkv_cm = tc.tile_pool(name="wkvpool", bufs=1)
            wkvpool = wkv_cm.__enter__()
            Wk_sb = wkvpool.tile([128, KT, npc], BF16)
            Wv_sb = wkvpool.tile([128, KT, npc], BF16)
            cst0 = stream_pool.tile([128, KT, 512], BF16, tag="stream")
            cT0_r = cT[:, 0:512].rearrange("(kt p) n -> p kt n", p=128)
            nc.sync.dma_start(bk_sb[:], bk[:, :])
            nc.sync.dma_start(bv_sb[:], bv[:, :])
            nc.gpsimd.partition_broadcast(BV[:], bv_sb[:], 128)
            # 4-kt pieces: big enough that HWDGE descriptor time (~0.6us per
            # dma_start) doesn't serialize the stream, small enough that the
            # first matmuls start after ~1MB instead of 4MB
            kp = 0
            for sz in (2, 2, 4, 4, 4):
                nc.scalar.dma_start(
                    Wk_sb[:, kp : kp + sz, :], wk_r[:, kp : kp + sz, :]
                )
                nc.sync.dma_start(
                    cst0[:, kp : kp + sz, :], cT0_r[:, kp : kp + sz, :]
                )
                kp += sz
            nc.scalar.dma_start(Wv_sb[:], wv_r[:, :, :])
            nc.scalar.dma_start(bq_sb[:], bq[:, :])

            # context side first: K^T and V (attention waits on these).
            # Chunk 0 runs kt-outer with one PSUM bank per head/slice so the
            # first matmul only waits on one kt-tile pair (~256KB), streaming
            # under the DMA instead of waiting for the full 4MB.
            for t in range(TCH):
                cst = cst0 if t == 0 else stream_chunk(cT, t)
                if t == 0:
                    psk = [psmm.tile([128, 512], F32, tag="mm512", name=f"psk{h}")
                           for h in range(hpc)]
                    for kt in range(KT):
                        for h in range(hpc):
                            nc.tensor.matmul(
                                psk[h][:],
                                Wk_sb[:, kt, ts(h, 128)],
                                cst[:, kt, :],
                                start=(kt == 0),
                                stop=(kt == KT - 1),
                            )
                    for h in range(hpc):
                        nc.scalar.add(
                            KTl[:, h, ts(t, 512)], psk[h][:], bk_sb[:, h : h + 1]
                        )
                    # V chunk 0 in two 2-bank passes: keeps 2 psmm bufs
                    # free so chunk-1 K can start while V copies drain
                    for jp in range(2):
                        psv = [psmm.tile([128, 512], F32, tag="mm512",
                                         name=f"psv{jp}{j}") for j in range(2)]
                        for kt in range(KT):
                            for j in range(2):
                                nc.tensor.matmul(
                                    psv[j][:, :npc],
                                    cst[:, kt, ts(2 * jp + j, 128)],
                                    Wv_sb[:, kt, :],
                                    start=(kt == 0),
                                    stop=(kt == KT - 1),
                                )
                        for j in range(2):
                            nc.vector.tensor_add(
                                V[:, t * 4 + 2 * jp + j, :], psv[j][:, :npc], BV[:]
                            )
                    continue
                for h in range(hpc):
                    ps = psmm.tile([128, 512], F32, tag="mm512")
                    for kt in range(KT):
                        nc.tensor.matmul(
                            ps[:],
                            Wk_sb[:, kt, ts(h, 128)],
                            cst[:, kt, :],
                            start=(kt == 0),
                            stop=(kt == KT - 1),
                        )
                    nc.scalar.add(KTl[:, h, ts(t, 512)], ps[:], bk_sb[:, h : h + 1])
                for j in range(4):
                    kt2 = t * 4 + j
                    ps = psmm.tile([128, 512], F32, tag="mm512")
                    for kt in range(KT):
                        nc.tensor.matmul(
                            ps[:, :npc],
                            cst[:, kt, ts(j, 128)],
                            Wv_sb[:, kt, :],
                            start=(kt == 0),
                            stop=(kt == KT - 1),
                        )
                    # bias add fused into the PSUM->SBUF copy (BV is bv
                    # broadcast across partitions; saves a K=1 matmul)
                    nc.vector.tensor_add(V[:, kt2, :], ps[:, :npc], BV[:])

            wkv_cm.__exit__(None, None, None)

            # Wq/Wp load behind the ctx streams: not needed until Q0/attention
            nc.scalar.dma_start(Wq_sb[:], wq_r[:, :, :])
            nc.scalar.dma_start(
                Wp_sb[:], wp[:, :].rearrange("(h p) d -> p h d", p=128)
            )

            xsts = {0: stream_chunk(xT, 0)}

            def do_q(t, h):
                ps = psmm.tile([128, 512], F32, tag="mm512", name="psq")
                for kt in range(KT):
                    nc.tensor.matmul(
                        ps[:],
                        Wq_sb[:, kt, ts(h, 128)],
                        xsts[t][:, kt, :],
                        start=(kt == 0),
                        stop=(kt == KT - 1),
                    )
                nc.scalar.add(QT[:, h, ts(t, 512)], ps[:], bq_sb[:, h : h + 1])

            for h in range(hpc):
                do_q(0, h)

            _attention(
                nc, tc, psmm, psov, psdn, small,
                QT, KTl, V, OT, Wp_sb, ones_col, out,
                dim, hpc, LT, TCH, scale, Exp,
                do_q, lambda t: xsts.__setitem__(t, stream_chunk(xT, t)),
            )


def _attention(
    nc, tc, psmm, psov, psdn, small,
    QT, KTl, V, OT, Wp_sb, ones_col, out,
    dim, hpc, LT, TCH, scale, Exp,
    do_q, prefetch_x,
):
        # ---- phase 2: attention + out-proj, software-pipelined ----
        # per (t,h): S^T matmuls + exp  ->  denominator (DVE fp16 add tree,
        # ones-matmul, reciprocal, partition broadcast)  ->  PV (V stationary,
        # P^T streams; O^T straight to PSUM)  ->  normalize-copy  ->  when a
        # token chunk's 4 heads are done, out-proj matmuls + bf16 DMA.
        with (
            tc.tile_pool(name="ptpool", bufs=3) as ppool,
            tc.tile_pool(name="treepool", bufs=1) as tpool,
            tc.tile_pool(name="rpool", bufs=2) as rpool,
        ):
            iters = [(t, h) for t in range(TCH) for h in range(hpc)]
            pts = {}
            rs = {}

            def do_st(i):
                t, h = iters[i]
                PT = ppool.tile([128, LT, 512], FP16, tag="pt")
                pts[i] = PT
                for kt2 in range(LT):
                    ps = psmm.tile([128, 512], F32, tag="mm512")
                    nc.tensor.matmul(
                        ps[:],
                        KTl[:, h, ts(kt2, 128)],
                        QT[:, h, ts(t, 512)],
                        start=True,
                        stop=True,
                    )
                    nc.scalar.activation(PT[:, kt2, :], ps[:], Exp, scale=scale)

            def do_dn(i):
                PT = pts[i]
                TR = tpool.tile([128, 15, 512], FP16, tag="tree")
                # each tree level is ONE strided DVE op over all pairs
                pt_pairs = PT[:, :, :].rearrange("p (a b) n -> p a b n", b=2)
                nc.vector.tensor_add(
                    TR[:, 0:8, :], pt_pairs[:, :, 0, :], pt_pairs[:, :, 1, :]
                )
                l1 = TR[:, 0:8, :].rearrange("p (a b) n -> p a b n", b=2)
                nc.vector.tensor_add(TR[:, 8:12, :], l1[:, :, 0, :], l1[:, :, 1, :])
                l2 = TR[:, 8:12, :].rearrange("p (a b) n -> p a b n", b=2)
                nc.vector.tensor_add(
                    TR[:, 12:14, :], l2[:, :, 0, :], l2[:, :, 1, :]
                )
                nc.vector.tensor_add(TR[:, 14, :], TR[:, 12, :], TR[:, 13, :])
                dn = psdn.tile([1, 512], F32, tag="dn")
                nc.tensor.matmul(
                    dn[:], ones_col[:], TR[:, 14, :], start=True, stop=True
                )
                rr = small.tile([1, 512], F32, tag="rr", bufs=2)
                nc.vector.reciprocal(rr[:], dn[:])
                R = rpool.tile([128, 512], F32, tag="R")
                nc.gpsimd.partition_broadcast(R[:], rr[:], 128)
                rs[i] = R

            def do_pv(i):
                t, h = iters[i]
                PT = pts.pop(i)
                R = rs.pop(i)
                po = psov.tile([128, 512], F32, tag="po")
                for kt2 in range(LT):
                    nc.tensor.matmul(
                        po[:],
                        V[:, kt2, ts(h, 128)],
                        PT[:, kt2, :],
                        start=(kt2 == 0),
                        stop=(kt2 == LT - 1),
                    )
                nc.vector.tensor_mul(OT[:, h, ts(t, 512)], po[:], R[:])

            def do_tail_group(tt, dc, drain=False):
                ps = psmm.tile([128, 512], F32, tag="mm512")
                for h in range(hpc):
                    nc.tensor.matmul(
                        ps[:],
                        OT[:, h, ts(tt, 128)],
                        Wp_sb[:, h, ts(dc, 512)],
                        start=(h == 0),
                        stop=(h == hpc - 1),
                    )
                ob = small.tile([128, 512], BF16, tag="ob")
                # during the final drain ACT is idle (no more exps): split the
                # PSUM->SBUF copies across both engines
                if drain and dc % 2 == 0:
                    nc.scalar.copy(ob[:], ps[:])
                else:
                    nc.vector.tensor_copy(ob[:], ps[:])
                nc.sync.dma_start(out[ts(tt, 128), ts(dc, 512)], ob[:])

            # dn lags S^T/exp by 1, PV by 2: ACT's exp stream and the DVE
            # tree never gate PE, and R is ready before each PV's normalize.
            # Each finished token chunk's 16 out-proj groups are spread 4 per
            # iteration so PE/DVE load stays uniform instead of bursting.
            n = len(iters)
            tail_q = []
            i = 0
            while i < n + 2 or tail_q:
                if i < n:
                    do_st(i)
                    ti, hi = iters[i]
                    # Q-projection for the next token chunk rides in this
                    # iteration's PE slack (attention is ACT/DVE-heavy)
                    if ti + 1 < TCH:
                        if hi == 0:
                            prefetch_x(ti + 1)
                        do_q(ti + 1, hi)
                if 0 <= i - 1 < n:
                    do_dn(i - 1)
                if 0 <= i - 2 < n:
                    do_pv(i - 2)
                    tp_, hp_ = iters[i - 2]
                    if hp_ == hpc - 1:
                        tail_q.extend(
                            (tp_ * 4 + j, dc)
                            for j in range(4)
                            for dc in range(dim // 512)
                        )
                for _ in range(4 if i < n else 8):
                    if tail_q:
                        do_tail_group(*tail_q.pop(0), drain=(i >= n))
                i += 1


def make_in_maps(x, context, W_qkv, b_qkv, W_proj):
    """Shard + pre-layout full inputs into per-core input maps."""
    x = np.asarray(x, dtype=np.float32)
    context = np.asarray(context, dtype=np.float32)
    W_qkv = np.asarray(W_qkv, dtype=np.float32)
    b_qkv = np.asarray(b_qkv, dtype=np.float32)
    W_proj = np.asarray(W_proj, dtype=np.float32)

    in_maps = []
    for c in range(N_CORES):
        b, g = divmod(c, GPB)
        n0 = g * NPC
        xTb = np.ascontiguousarray(x[b].T).astype(BF16_NP)
        cTb = np.ascontiguousarray(context[b].T).astype(BF16_NP)
        in_maps.append(
            {
                "xT": xTb,
                "cT": cTb,
                "wq": np.ascontiguousarray(W_qkv[:, n0 : n0 + NPC]).astype(BF16_NP),
                "wk": np.ascontiguousarray(
                    W_qkv[:, DIM + n0 : DIM + n0 + NPC]
                ).astype(BF16_NP),
                "wv": np.ascontiguousarray(
                    W_qkv[:, 2 * DIM + n0 : 2 * DIM + n0 + NPC]
                ).astype(BF16_NP),
                "bq": np.ascontiguousarray(
                    b_qkv[n0 : n0 + NPC].reshape(HPC, 128).T
                ).astype(np.float32),
                "bk": np.ascontiguousarray(
                    b_qkv[DIM + n0 : DIM + n0 + NPC].reshape(HPC, 128).T
                ).astype(np.float32),
                "bv": np.ascontiguousarray(
                    b_qkv[2 * DIM + n0 : 2 * DIM + n0 + NPC].reshape(1, NPC)
                ).astype(np.float32),
                "wp": np.ascontiguousarray(W_proj[n0 : n0 + NPC, :]).astype(BF16_NP),
            }
        )
    return in_maps


_NC_CACHE = {}


def kernel(x, context, W_qkv, b_qkv, W_proj, b_proj, _trace=False):
    from concourse.bass_utils import run_bass_kernel_spmd

    b_proj = np.asarray(b_proj, dtype=np.float32)
    in_maps = make_in_maps(x, context, W_qkv, b_qkv, W_proj)

    if "nc" not in _NC_CACHE:
        _NC_CACHE["nc"] = build_bass()
    nc = _NC_CACHE["nc"]

    res = run_bass_kernel_spmd(nc, in_maps, list(range(N_CORES)), trace=_trace)
    results = res.results

    out = np.zeros((B, L, DIM), dtype=np.float32)
    for c in range(N_CORES):
        b = c // GPB
        out[b] += results[c]["out"].astype(np.float32)
    out += b_proj[None, None, :]
    if _trace:
        return out, res
    return out


# revision 32
# speedup vs baseline: 1.0064x; 1.0064x over previous
"""Cross-attention Trainium2 kernel, SPMD over 8 NeuronCores.

Sharding: core c = b*4 + g handles batch b (of 2) and head-group g (of 4,
4 heads each) — data parallel on B, Megatron tensor parallel on heads:
W_qkv column-sliced, W_proj row-sliced, partial outputs summed on host.

Per-core dataflow (all matmuls 16-bit operands, fp32 PSUM accumulation):
  - host pre-transposes x/context to [D, L] and pre-casts weights to bf16
  - Q^T[n,tok] = Wq.T @ x^T      (lhsT=Wq tile, rhs=x^T tile, + bias via ACT)
  - K^T[n,key] = Wk.T @ c^T      (same)
  - V[key,n]   = c @ Wv          (lhsT=c^T tile, rhs=Wv, bias via K=1 matmul)
  - S^T[key,tok] = K_h @ Q_h^T   (one matmul per tile, contraction=head_dim)
  - P^T = exp(scale * S^T)       (ACT, fp16 out; no max subtraction: scores
                                  ~ N(0,1) so exp never overflows)
  - O^T[hd,tok] = V_h.T @ P^T    (lhsT=V tile (stationary), rhs=P^T streams
                                  N=512 — avoids per-matmul LDW stalls and
                                  produces O^T directly: no PE transposes)
  - denom[tok] = ones.T @ (tree-sum of P^T tiles)  (DVE fp16 add tree +
                                  one K=128 matmul; reciprocal on DVE;
                                  broadcast across partitions on GPSIMD)
  - OT = O^T * recip(denom)      (DVE multiply fused with PSUM->SBUF copy)
  - out[tok,dout] = O @ Wp       (lhsT=O^T tile, rhs=Wp tile; bf16 DMA out)
"""

import numpy as np
import ml_dtypes

import concourse.bass as bass
import concourse.bacc as bacc
import concourse.mybir as mybir
from concourse import bass_isa
from concourse.bass import ts
from concourse.tile import TileContext

DIM = 2048
NUM_HEADS = 16
HEAD_DIM = 128
B, L = 2, 2048
GPB = 4                  # head-groups per batch (cores per batch)
HPC = NUM_HEADS // GPB   # heads per core = 4
NPC = HPC * HEAD_DIM     # per-core projection width = 512
N_CORES = 8

F32 = mybir.dt.float32
BF16 = mybir.dt.bfloat16
FP16 = mybir.dt.float16
BF16_NP = ml_dtypes.bfloat16


def build_bass(dim=DIM, seq=L, hpc=HPC, hd=HEAD_DIM, repeat=1):
    """Build the per-core SPMD Bass program (parameterized for testing)."""
    npc = hpc * hd
    KT = dim // 128      # contraction tiles over model dim
    LT = seq // 128      # token/key 128-tiles
    TCH = seq // 512     # token 512-chunks
    scale = float(hd) ** -0.5
    Exp = mybir.ActivationFunctionType.Exp

    nc = bacc.Bacc()
    xT = nc.dram_tensor("xT", [dim, seq], BF16, kind="ExternalInput")
    cT = nc.dram_tensor("cT", [dim, seq], BF16, kind="ExternalInput")
    wq = nc.dram_tensor("wq", [dim, npc], BF16, kind="ExternalInput")
    wk = nc.dram_tensor("wk", [dim, npc], BF16, kind="ExternalInput")
    wv = nc.dram_tensor("wv", [dim, npc], BF16, kind="ExternalInput")
    bq = nc.dram_tensor("bq", [128, hpc], F32, kind="ExternalInput")
    bk = nc.dram_tensor("bk", [128, hpc], F32, kind="ExternalInput")
    bv = nc.dram_tensor("bv", [1, npc], F32, kind="ExternalInput")
    wp = nc.dram_tensor("wp", [npc, dim], BF16, kind="ExternalInput")
    out = nc.dram_tensor("out", [seq, dim], BF16, kind="ExternalOutput")

    with TileContext(nc) as tc:
        with (
            tc.tile_pool(name="psmm", bufs=6, space="PSUM") as psmm,
            tc.tile_pool(name="psov", bufs=2, space="PSUM") as psov,
            tc.tile_pool(name="small", bufs=3) as small,
        ):
            for _rep in range(repeat):
                _build_body(
                    nc, tc, psmm, psov, small,
                    xT, cT, wq, wk, wv, bq, bk, bv, wp, out,
                    dim, seq, hpc, hd, npc, KT, LT, TCH, scale, Exp,
                )

    nc.compile()
    return nc


def _build_body(
    nc, tc, psmm, psov, small,
    xT, cT, wq, wk, wv, bq, bk, bv, wp, out,
    dim, seq, hpc, hd, npc, KT, LT, TCH, scale, Exp,
):
    with tc.tile_pool(name="res", bufs=1) as res:
        QT = res.tile([128, hpc, seq], BF16)    # [hd, h, tok]
        KTl = res.tile([128, hpc, seq], BF16)   # [hd, h, key]
        V = res.tile([128, LT, npc], BF16)      # [key, ktile, h*hd]
        OT = res.tile([128, hpc, seq], BF16)    # [hd, h, tok] normalized
        Wp_sb = res.tile([128, hpc, dim], BF16)
        bq_sb = res.tile([128, hpc], F32)
        bk_sb = res.tile([128, hpc], F32)
        bv_sb = res.tile([1, npc], F32)
        ones_col = res.tile([128, 1], FP16)     # denominator matmul stationary
        BV = res.tile([128, npc], F32)          # bv broadcast across partitions

        nc.vector.memset(ones_col[:], 1.0)

        # ---- phase 1: projections ----
        # stream + Wq pools stay alive through attention (Q chunks 1-3 are
        # computed inside the attention loop, in PE slack); Wk/Wv free early.
        with (
            tc.tile_pool(name="stream", bufs=2) as stream_pool,
            tc.tile_pool(name="wqpool", bufs=1) as wqpool,
        ):
            Wq_sb = wqpool.tile([128, KT, npc], BF16)
            wk_r = wk[:, :].rearrange("(kt p) n -> p kt n", p=128)
            wv_r = wv[:, :].rearrange("(kt p) n -> p kt n", p=128)
            wq_r = wq[:, :].rearrange("(kt p) n -> p kt n", p=128)

            def stream_chunk(src, t):
                st_tile = stream_pool.tile([128, KT, 512], BF16, tag="stream")
                src_r = src[:, ts(t, 512)].rearrange("(kt p) n -> p kt n", p=128)
                nc.sync.dma_start(st_tile[:], src_r[:, :, :])
                return st_tile

            # load order: sync queue carries biases + the ctx/x token streams,
            # scalar (ACT hwdge) queue carries the big weights — the two
            # queues drain concurrently so chunk-0 compute isn't serialized
            # behind 4MB of weights
            wkv_cm = tc.tile_pool(name="wkvpool", bufs=1)
            wkvpool = wkv_cm.__enter__()
            Wk_sb = wkvpool.tile([128, KT, npc], BF16)
            Wv_sb = wkvpool.tile([128, KT, npc], BF16)
            cst0 = stream_pool.tile([128, KT, 512], BF16, tag="stream")
            cT0_r = cT[:, 0:512].rearrange("(kt p) n -> p kt n", p=128)
            nc.sync.dma_start(bk_sb[:], bk[:, :])
            nc.sync.dma_start(bv_sb[:], bv[:, :])
            nc.gpsimd.partition_broadcast(BV[:], bv_sb[:], 128)
            # 4-kt pieces: big enough that HWDGE descriptor time (~0.6us per
            # dma_start) doesn't serialize the stream, small enough that the
            # first matmuls start after ~1MB instead of 4MB
            kp = 0
            for sz in (2, 2, 4, 4, 4):
                nc.scalar.dma_start(
                    Wk_sb[:, kp : kp + sz, :], wk_r[:, kp : kp + sz, :]
                )
                nc.sync.dma_start(
                    cst0[:, kp : kp + sz, :], cT0_r[:, kp : kp + sz, :]
                )
                kp += sz
            nc.scalar.dma_start(Wv_sb[:], wv_r[:, :, :])
            nc.scalar.dma_start(bq_sb[:], bq[:, :])

            # context side first: K^T and V (attention waits on these).
            # Chunk 0 runs kt-outer with one PSUM bank per head/slice so the
            # first matmul only waits on one kt-tile pair (~256KB), streaming
            # under the DMA instead of waiting for the full 4MB.
            for t in range(TCH):
                cst = cst0 if t == 0 else stream_chunk(cT, t)
                if t == 0:
                    psk = [psmm.tile([128, 512], F32, tag="mm512", name=f"psk{h}")
                           for h in range(hpc)]
                    for kt in range(KT):
                        for h in range(hpc):
                            nc.tensor.matmul(
                                psk[h][:],
                                Wk_sb[:, kt, ts(h, 128)],
                                cst[:, kt, :],
                                start=(kt == 0),
                                stop=(kt == KT - 1),
                            )
                    for h in range(hpc):
                        nc.scalar.add(
                            KTl[:, h, ts(t, 512)], psk[h][:], bk_sb[:, h : h + 1]
                        )
                    # V chunk 0 in two 2-bank passes: keeps 2 psmm bufs
                    # free so chunk-1 K can start while V copies drain
                    for jp in range(2):
                        psv = [psmm.tile([128, 512], F32, tag="mm512",
                                         name=f"psv{jp}{j}") for j in range(2)]
                        for kt in range(KT):
                            for j in range(2):
                                nc.tensor.matmul(
                                    psv[j][:, :npc],
                                    cst[:, kt, ts(2 * jp + j, 128)],
                                    Wv_sb[:, kt, :],
                                    start=(kt == 0),
                                    stop=(kt == KT - 1),
                                )
                        for j in range(2):
                            nc.vector.tensor_add(
                                V[:, t * 4 + 2 * jp + j, :], psv[j][:, :npc], BV[:]
                            )
                    continue
                for h in range(hpc):
                    ps = psmm.tile([128, 512], F32, tag="mm512")
                    for kt in range(KT):
                        nc.tensor.matmul(
                            ps[:],
                            Wk_sb[:, kt, ts(h, 128)],
                            cst[:, kt, :],
                            start=(kt == 0),
                            stop=(kt == KT - 1),
                        )
                    nc.scalar.add(KTl[:, h, ts(t, 512)], ps[:], bk_sb[:, h : h + 1])
                for j in range(4):
                    kt2 = t * 4 + j
                    ps = psmm.tile([128, 512], F32, tag="mm512")
                    for kt in range(KT):
                        nc.tensor.matmul(
                            ps[:, :npc],
                            cst[:, kt, ts(j, 128)],
                            Wv_sb[:, kt, :],
                            start=(kt == 0),
                            stop=(kt == KT - 1),
                        )
                    # bias add fused into the PSUM->SBUF copy (BV is bv
                    # broadcast across partitions; saves a K=1 matmul)
                    nc.vector.tensor_add(V[:, kt2, :], ps[:, :npc], BV[:])

            wkv_cm.__exit__(None, None, None)

            # Wq/Wp load behind the ctx streams: not needed until Q0/attention
            nc.scalar.dma_start(Wq_sb[:], wq_r[:, :, :])
            nc.scalar.dma_start(
                Wp_sb[:], wp[:, :].rearrange("(h p) d -> p h d", p=128)
            )

            xsts = {0: stream_chunk(xT, 0)}

            def do_q(t, h):
                ps = psmm.tile([128, 512], F32, tag="mm512", name="psq")
                for kt in range(KT):
                    nc.tensor.matmul(
                        ps[:],
                        Wq_sb[:, kt, ts(h, 128)],
                        xsts[t][:, kt, :],
                        start=(kt == 0),
                        stop=(kt == KT - 1),
                    )
                nc.scalar.add(QT[:, h, ts(t, 512)], ps[:], bq_sb[:, h : h + 1])

            for h in range(hpc):
                do_q(0, h)

            _attention(
                nc, tc, psmm, psov, small,
                QT, KTl, V, OT, Wp_sb, ones_col, out,
                dim, hpc, LT, TCH, scale, Exp,
                do_q, lambda t: xsts.__setitem__(t, stream_chunk(xT, t)),
            )


def _attention(
    nc, tc, psmm, psov, small,
    QT, KTl, V, OT, Wp_sb, ones_col, out,
    dim, hpc, LT, TCH, scale, Exp,
    do_q, prefetch_x,
):
        # ---- phase 2: attention + out-proj, software-pipelined ----
        # per (t,h): S^T matmuls + exp  ->  denominator (DVE fp16 add tree,
        # ones-matmul, reciprocal, partition broadcast)  ->  PV (V stationary,
        # P^T streams; O^T straight to PSUM)  ->  normalize-copy  ->  when a
        # token chunk's 4 heads are done, out-proj matmuls + bf16 DMA.
        with (
            tc.tile_pool(name="ptpool", bufs=3) as ppool,
            tc.tile_pool(name="treepool", bufs=1) as tpool,
            tc.tile_pool(name="rpool", bufs=2) as rpool,
        ):
            iters = [(t, h) for t in range(TCH) for h in range(hpc)]
            pts = {}
            rs = {}

            def do_st(i):
                t, h = iters[i]
                PT = ppool.tile([128, LT, 512], FP16, tag="pt")
                pts[i] = PT
                for kt2 in range(LT):
                    ps = psmm.tile([128, 512], F32, tag="mm512")
                    nc.tensor.matmul(
                        ps[:],
                        KTl[:, h, ts(kt2, 128)],
                        QT[:, h, ts(t, 512)],
                        start=True,
                        stop=True,
                    )
                    nc.scalar.activation(PT[:, kt2, :], ps[:], Exp, scale=scale)

            def do_dn(i):
                PT = pts[i]
                TR = tpool.tile([128, 15, 512], FP16, tag="tree")
                # each tree level is ONE strided DVE op over all pairs
                pt_pairs = PT[:, :, :].rearrange("p (a b) n -> p a b n", b=2)
                nc.vector.tensor_add(
                    TR[:, 0:8, :], pt_pairs[:, :, 0, :], pt_pairs[:, :, 1, :]
                )
                l1 = TR[:, 0:8, :].rearrange("p (a b) n -> p a b n", b=2)
                nc.vector.tensor_add(TR[:, 8:12, :], l1[:, :, 0, :], l1[:, :, 1, :])
                l2 = TR[:, 8:12, :].rearrange("p (a b) n -> p a b n", b=2)
                nc.vector.tensor_add(
                    TR[:, 12:14, :], l2[:, :, 0, :], l2[:, :, 1, :]
                )
                nc.vector.tensor_add(TR[:, 14, :], TR[:, 12, :], TR[:, 13, :])
                # denominator finish on GPSIMD: all-reduce over the key
                # partitions lands the broadcast sums directly; reciprocal
                # in place on DVE. Saves a PE matmul and a PSUM bank.
                R = rpool.tile([128, 512], F32, tag="R")
                nc.gpsimd.partition_all_reduce(
                    R[:], TR[:, 14, :], 128, bass_isa.ReduceOp.add
                )
                nc.vector.reciprocal(R[:], R[:])
                rs[i] = R

            def do_pv(i):
                t, h = iters[i]
                PT = pts.pop(i)
                R = rs.pop(i)
                po = psov.tile([128, 512], F32, tag="po")
                for kt2 in range(LT):
                    nc.tensor.matmul(
                        po[:],
                        V[:, kt2, ts(h, 128)],
                        PT[:, kt2, :],
                        start=(kt2 == 0),
                        stop=(kt2 == LT - 1),
                    )
                nc.vector.tensor_mul(OT[:, h, ts(t, 512)], po[:], R[:])

            def do_tail_group(tt, dc, drain=False):
                ps = psmm.tile([128, 512], F32, tag="mm512")
                for h in range(hpc):
                    nc.tensor.matmul(
                        ps[:],
                        OT[:, h, ts(tt, 128)],
                        Wp_sb[:, h, ts(dc, 512)],
                        start=(h == 0),
                        stop=(h == hpc - 1),
                    )
                ob = small.tile([128, 512], BF16, tag="ob")
                # during the final drain ACT is idle (no more exps): split the
                # PSUM->SBUF copies across both engines
                if drain and dc % 2 == 0:
                    nc.scalar.copy(ob[:], ps[:])
                else:
                    nc.vector.tensor_copy(ob[:], ps[:])
                nc.sync.dma_start(out[ts(tt, 128), ts(dc, 512)], ob[:])

            # dn lags S^T/exp by 1, PV by 2: ACT's exp stream and the DVE
            # tree never gate PE, and R is ready before each PV's normalize.
            # Each finished token chunk's 16 out-proj groups are spread 4 per
            # iteration so PE/DVE load stays uniform instead of bursting.
            n = len(iters)
            tail_q = []
            i = 0
            while i < n + 2 or tail_q:
                if i < n:
                    do_st(i)
                    ti, hi = iters[i]
                    # Q-projection for the next token chunk rides in this
                    # iteration's PE slack (attention is ACT/DVE-heavy)
                    if ti + 1 < TCH:
                        if hi == 0:
                            prefetch_x(ti + 1)
                        do_q(ti + 1, hi)
                if 0 <= i - 1 < n:
                    do_dn(i - 1)
                if 0 <= i - 2 < n:
                    do_pv(i - 2)
                    tp_, hp_ = iters[i - 2]
                    if hp_ == hpc - 1:
                        tail_q.extend(
                            (tp_ * 4 + j, dc)
                            for j in range(4)
                            for dc in range(dim // 512)
                        )
                for _ in range(4 if i < n else 8):
                    if tail_q:
                        do_tail_group(*tail_q.pop(0), drain=(i >= n))
                i += 1


def make_in_maps(x, context, W_qkv, b_qkv, W_proj):
    """Shard + pre-layout full inputs into per-core input maps."""
    x = np.asarray(x, dtype=np.float32)
    context = np.asarray(context, dtype=np.float32)
    W_qkv = np.asarray(W_qkv, dtype=np.float32)
    b_qkv = np.asarray(b_qkv, dtype=np.float32)
    W_proj = np.asarray(W_proj, dtype=np.float32)

    in_maps = []
    for c in range(N_CORES):
        b, g = divmod(c, GPB)
        n0 = g * NPC
        xTb = np.ascontiguousarray(x[b].T).astype(BF16_NP)
        cTb = np.ascontiguousarray(context[b].T).astype(BF16_NP)
        in_maps.append(
            {
                "xT": xTb,
                "cT": cTb,
                "wq": np.ascontiguousarray(W_qkv[:, n0 : n0 + NPC]).astype(BF16_NP),
                "wk": np.ascontiguousarray(
                    W_qkv[:, DIM + n0 : DIM + n0 + NPC]
                ).astype(BF16_NP),
                "wv": np.ascontiguousarray(
                    W_qkv[:, 2 * DIM + n0 : 2 * DIM + n0 + NPC]
                ).astype(BF16_NP),
                "bq": np.ascontiguousarray(
                    b_qkv[n0 : n0 + NPC].reshape(HPC, 128).T
                ).astype(np.float32),
                "bk": np.ascontiguousarray(
                    b_qkv[DIM + n0 : DIM + n0 + NPC].reshape(HPC, 128).T
                ).astype(np.float32),
                "bv": np.ascontiguousarray(
                    b_qkv[2 * DIM + n0 : 2 * DIM + n0 + NPC].reshape(1, NPC)
                ).astype(np.float32),
                "wp": np.ascontiguousarray(W_proj[n0 : n0 + NPC, :]).astype(BF16_NP),
            }
        )
    return in_maps


_NC_CACHE = {}


def kernel(x, context, W_qkv, b_qkv, W_proj, b_proj, _trace=False):
    from concourse.bass_utils import run_bass_kernel_spmd

    b_proj = np.asarray(b_proj, dtype=np.float32)
    in_maps = make_in_maps(x, context, W_qkv, b_qkv, W_proj)

    if "nc" not in _NC_CACHE:
        _NC_CACHE["nc"] = build_bass()
    nc = _NC_CACHE["nc"]

    res = run_bass_kernel_spmd(nc, in_maps, list(range(N_CORES)), trace=_trace)
    results = res.results

    out = np.zeros((B, L, DIM), dtype=np.float32)
    for c in range(N_CORES):
        b = c // GPB
        out[b] += results[c]["out"].astype(np.float32)
    out += b_proj[None, None, :]
    if _trace:
        return out, res
    return out
